# revision 17
# baseline (speedup 1.0000x reference)
"""GQA attention (b=2, s=2048, d=2048, H=16, Hkv=4, depth=128) on 8 trn2 cores.

Sharding: core c = 4*b + j (b in {0,1}, j in {0..3}) handles batch b and
q-heads {2j, 2j+1, 2j+8, 2j+9}.  This model's RoPE rotates the full projected
vector (pairing dim i with i + d/2), so roped q-head h mixes raw column
blocks {h mod 8, (h mod 8) + 8}; the head grouping above makes the Wq column
shard exactly 512 columns with no duplication.  Those q-heads attend kv-heads
{g0, g0+2} (g0 = 0 for j<2 else 1), which likewise pair up under RoPE.
Each core of a pair projects ONE raw k block and ONE v head; the pair swaps
them with a 2-way AllGather, halving the duplicated K/V projection work.
Wo is row-sharded over the 4 local head-dims; the 4 per-batch bf16 partials
are summed on the host (fp32) and bo added.

Device layout is fully transposed (feature dim on partitions): q_r^T, k_r^T
are [depth, s]; logits l^T = k_r^T.T @ q_r^T so softmax's reduce axis is on
partitions and PV needs no transposes.  All matmuls bf16 (fp32 PSUM).

Schedule (PE issue order == PE execution order):
  [KV proj][Q proj i=0,1][attends half-0 rounds h0..h3]
  [attends half-1 rounds h0..h3, out-proj for m 0..7 woven into the skt
   loops two matmuls at a time][out-proj tail m 8..15]
Attention softmax denominators: running bf16 sums on DVE (2x mode), one
ones-matmul partition-reduce + one broadcast matmul per (half, st) on PE,
reciprocal as exp(-ln(d)) on ACT over the broadcast tile.  o PSUM banks are
drained to SBUF f32 immediately (ACT) so the next half's PV can start.
"""
import numpy as np
import ml_dtypes
from contextlib import ExitStack

import concourse.bass as bass
import concourse.mybir as mybir
import concourse.tile as tile
from concourse.bass import ts
from concourse.bass_utils import run_bass_kernel_spmd

BF = mybir.dt.bfloat16
F32 = mybir.dt.float32
NPBF = ml_dtypes.bfloat16

S = 2048          # sequence length
D = 2048          # d_model
DEPTH = 128       # head dim
NKC = 16          # contraction chunks of 128 over d_model
NST = 4           # 512-wide s tiles
INV_SQRT_D = 1.0 / float(np.sqrt(np.float32(DEPTH)))

_NC_CACHE = None
LAST_RESULT = None  # BassKernelResults of the most recent run (for profiling)


def _split_waits(nc, limit=1):
    """walrus rejects instructions carrying more than a couple of sem waits
    ('Too many sync wait commands').  Move excess waits onto dedicated NoOps
    on the same engine, placed immediately before the instruction."""
    idx = 0
    for f in nc.m.functions:
        for blk in f.blocks:
            insts = blk.instructions
            out = []
            for inst in insts:
                si = inst.sync_info
                if si is not None and len(si.on_wait) > limit:
                    waits = list(si.on_wait)
                    extra, keep = waits[:-limit], waits[-limit:]
                    for w in extra:
                        nop = mybir.InstNoOp(name=f"waitsplit_{idx}", ins=[], outs=[])
                        idx += 1
                        nop.engine = inst.engine
                        nop.bass_nofuse = True
                        nop.sync_info = mybir.SyncInfo(on_wait=[w], on_update=[])
                        out.append(nop)
                    inst.sync_info = mybir.SyncInfo(
                        on_wait=keep, on_update=list(si.on_update)
                    )
                out.append(inst)
            insts[:] = out


def _build_nc():
    nc = bass.Bass(num_devices=8)
    xT = nc.dram_tensor("xT", [128, NKC, S], BF, kind="ExternalInput")
    wq = nc.dram_tensor("wq", [128, NKC, 512], BF, kind="ExternalInput")
    wk = nc.dram_tensor("wk", [128, NKC, 128], BF, kind="ExternalInput")
    wv = nc.dram_tensor("wv", [128, NKC, 128], BF, kind="ExternalInput")
    wo = nc.dram_tensor("wo", [128, 4, D], BF, kind="ExternalInput")
    cq = nc.dram_tensor("cq", [128, 2, S], BF, kind="ExternalInput")
    sq = nc.dram_tensor("sq", [128, 2, S], BF, kind="ExternalInput")
    ck = nc.dram_tensor("ck", [128, S], BF, kind="ExternalInput")
    sk = nc.dram_tensor("sk", [128, S], BF, kind="ExternalInput")
    out = nc.dram_tensor("out", [128, 16, D], BF, kind="ExternalOutput")

    with tile.TileContext(nc) as tc, ExitStack() as top:
        pool_p = top.enter_context(tc.tile_pool(name="persist", bufs=1))
        # PSUM: 4 banks logits (double-buffered [128,1024]) + 2 banks o
        # accumulators + 2 rotating banks (den/bcast/out-proj groups)
        pp_lg = top.enter_context(tc.tile_pool(name="pslg", bufs=2, space="PSUM"))
        pp_o = top.enter_context(tc.tile_pool(name="pso", bufs=2, space="PSUM"))
        pp_rot = top.enter_context(tc.tile_pool(name="psrot", bufs=2, space="PSUM"))

        qr = pool_p.tile([128, 4, S], BF)        # roped qT, slots [a0,a1,a0+8,a1+8]
        kr = pool_p.tile([128, 2, S], BF)        # roped kT,  slots [g0, g0+2]
        vn = pool_p.tile([128, 2, NKC, DEPTH], BF)  # v native [p, g, skc, dv]
        wo_sb = pool_p.tile([128, 4, D], BF)
        ones_col = pool_p.tile([128, 1], BF)
        ones_row = pool_p.tile([1, 128], BF)
        nc.vector.memset(ones_col[:], 1.0)
        nc.vector.memset(ones_row[:], 1.0)

        # ---------------- phase A: projections + rope -----------------
        with ExitStack() as pA:
            pool_x = pA.enter_context(tc.tile_pool(name="pax", bufs=16))
            pool_w = pA.enter_context(tc.tile_pool(name="paw", bufs=16))
            pool_tab = pA.enter_context(tc.tile_pool(name="pat", bufs=1))
            pool_raw = pA.enter_context(tc.tile_pool(name="parw", bufs=2))
            pool_kv = pA.enter_context(tc.tile_pool(name="pakv", bufs=1))
            pool_dram = pA.enter_context(tc.tile_pool(name="padr", bufs=1, space="DRAM"))

            # DMA issue queues: Sync gets x chunks + wq + tables (+ later the
            # collective returns and v transposes); Scalar gets wk/wv and the
            # kv-exchange send so the collective fires as early as possible.
            xTs, wqs, wks, wvs = [], [], [], []
            for kc in range(NKC):
                xt_t = pool_x.tile([128, S], BF, tag="xt", name=f"xt_{kc}")
                nc.sync.dma_start(xt_t[:], xT[:, kc, :])
                xTs.append(xt_t)
                wk_t = pool_w.tile([128, 128], BF, tag="wk", name=f"wk_{kc}")
                nc.scalar.dma_start(wk_t[:], wk[:, kc, :])
                wks.append(wk_t)
            for kc in range(NKC):
                wv_t = pool_w.tile([128, 128], BF, tag="wv", name=f"wv_{kc}")
                nc.scalar.dma_start(wv_t[:], wv[:, kc, :])
                wvs.append(wv_t)
            for kc in range(NKC):
                wq_t = pool_w.tile([128, 512], BF, tag="wq", name=f"wq_{kc}")
                nc.sync.dma_start(wq_t[:], wq[:, kc, :])
                wqs.append(wq_t)
            ck_sb = pool_tab.tile([128, S], BF)
            nc.sync.dma_start(ck_sb[:], ck[:])
            sk_sb = pool_tab.tile([128, S], BF)
            nc.sync.dma_start(sk_sb[:], sk[:])
            cq_sb = pool_tab.tile([128, 2, S], BF)
            sq_sb = pool_tab.tile([128, 2, S], BF)
            nc.sync.dma_start(cq_sb[:, 0, :], cq[:, 0, :])
            nc.sync.dma_start(sq_sb[:, 0, :], sq[:, 0, :])
            nc.sync.dma_start(cq_sb[:, 1, :], cq[:, 1, :])
            nc.sync.dma_start(sq_sb[:, 1, :], sq[:, 1, :])
            nc.sync.dma_start(wo_sb[:], wo[:])

            def proj_4st(w_of, drain_to):
                """contract over all 16 kc into 4 [128,512] st accumulators
                spread over all three PSUM pools (so consecutive calls
                double-buffer), then drain via ACT copies.
                w_of(kc) -> stationary AP; drain_to(st) -> SBUF AP."""
                acc_lg = pp_lg.tile([128, 1024], F32, tag="lg", name="acc_lg")
                acc_o = pp_o.tile([128, 512], F32, tag="o", name="acc_o")
                acc_r = pp_rot.tile([128, 512], F32, tag="rot", name="acc_r")
                accs = [acc_lg[:, 0:512], acc_lg[:, 512:1024], acc_o[:], acc_r[:]]
                for kc in range(NKC):
                    for st in range(NST):
                        nc.tensor.matmul(
                            accs[st],
                            w_of(kc),
                            xTs[kc][:, ts(st, 512)],
                            start=(kc == 0),
                            stop=(kc == NKC - 1),
                        )
                for st in range(NST):
                    nc.vector.tensor_copy(drain_to(st), accs[st])

            # K/V: each core of a pair projects ONE raw k block and ONE v
            # head; the pair exchanges them with an AllGather.
            kv_sb = pool_kv.tile([128, 2 * S], BF, tag="kvmine")
            proj_4st(lambda kc: wks[kc][:], lambda st: kv_sb[:, ts(st, 512)])
            proj_4st(lambda kc: wvs[kc][:], lambda st: kv_sb[:, ts(NST + st, 512)])
            kv_in = pool_dram.tile([128, 2 * S], BF)
            kv_out = pool_dram.tile([2, 128, 2 * S], BF)
            nc.scalar.dma_start(kv_in[:], kv_sb[:])
            nc.gpsimd.collective_compute(
                "AllGather",
                mybir.AluOpType.bypass,
                replica_groups=[[0, 1], [2, 3], [4, 5], [6, 7]],
                ins=[kv_in.opt()],
                outs=[kv_out.opt()],
            )
            kboth = pool_kv.tile([128, 2, S], BF, tag="kboth")
            vtboth = pool_kv.tile([128, 2, S], BF, tag="vtboth")
            for r in range(2):
                nc.sync.dma_start(kboth[:, r, :], kv_out[r, :, 0:S])
                nc.sync.dma_start(vtboth[:, r, :], kv_out[r, :, S:2 * S])
            # v native via DMA transpose (g=0 needed first)
            for g in range(2):
                for skt in range(NKC):
                    nc.sync.dma_start_transpose(
                        vn[:, g, skt, :], vtboth[:, g, ts(skt, 128)]
                    )

            def rope_pair(x1_of, x2_of, c_of, s_of, out1_of, out2_of):
                """roped = (x1*c - x2*s, x2*c + x1*s), [128,1024]-wide DVE ops."""
                for h2 in range(2):  # S in 1024 halves
                    sl = ts(h2, 1024)
                    x1, x2 = x1_of(sl), x2_of(sl)
                    c_ap, s_ap = c_of(sl), s_of(sl)
                    t1 = pool_raw.tile([128, 1024], BF, tag="rt1", bufs=1, name="t1")
                    t2 = pool_raw.tile([128, 1024], BF, tag="rt2", bufs=1, name="t2")
                    nc.vector.tensor_mul(t1[:], x1, c_ap)
                    nc.vector.tensor_mul(t2[:], x2, s_ap)
                    nc.vector.tensor_sub(out1_of(sl), t1[:], t2[:])
                    t3 = pool_raw.tile([128, 1024], BF, tag="rt1", bufs=1, name="t3")
                    t4 = pool_raw.tile([128, 1024], BF, tag="rt2", bufs=1, name="t4")
                    nc.vector.tensor_mul(t3[:], x2, c_ap)
                    nc.vector.tensor_mul(t4[:], x1, s_ap)
                    nc.vector.tensor_add(out2_of(sl), t3[:], t4[:])

            # Q raw pair (i, 2+i) -> roped qr slots (i, 2+i).  The k rope is
            # emitted between i=0 and i=1 so the DVE FIFO doesn't block the
            # early q ropes on the collective's arrival.
            for i in range(2):
                raw1 = pool_raw.tile([128, S], BF, tag="raw1", bufs=2, name=f"q1_{i}")
                raw2 = pool_raw.tile([128, S], BF, tag="raw2", bufs=2, name=f"q2_{i}")
                proj_4st(lambda kc: wqs[kc][:, ts(i, 128)],
                         lambda st: raw1[:, ts(st, 512)])
                proj_4st(lambda kc: wqs[kc][:, ts(2 + i, 128)],
                         lambda st: raw2[:, ts(st, 512)])
                rope_pair(lambda sl: raw1[:, sl], lambda sl: raw2[:, sl],
                          lambda sl: cq_sb[:, i, sl], lambda sl: sq_sb[:, i, sl],
                          lambda sl: qr[:, i, sl], lambda sl: qr[:, 2 + i, sl])
                if i == 0:
                    rope_pair(lambda sl: kboth[:, 0, sl],
                              lambda sl: kboth[:, 1, sl],
                              lambda sl: ck_sb[:, sl], lambda sl: sk_sb[:, sl],
                              lambda sl: kr[:, 0, sl], lambda sl: kr[:, 1, sl])

        # ------------- phase B: attention + output projection -------------
        with ExitStack() as pB:
            pool_e = pB.enter_context(tc.tile_pool(name="pe", bufs=8))
            pool_sums = pB.enter_context(tc.tile_pool(name="psum_s", bufs=2))
            pool_of = pB.enter_context(tc.tile_pool(name="pof", bufs=2))
            pool_db = pB.enter_context(tc.tile_pool(name="pdb", bufs=4))
            pool_osb = pB.enter_context(tc.tile_pool(name="posb", bufs=3))
            pool_on = pB.enter_context(tc.tile_pool(name="pon", bufs=1))
            onorm = pool_on.tile([128, 4, S], BF)  # normalized o^T per head

            class OutProj:
                """Emits the output projection for m-blocks (128 rows of sq)
                one ct-pair at a time (4 hi x 2 ct accumulating matmuls into
                2 rotating PSUM banks + drains) so it can be woven into
                attend skt loops."""
                def __init__(self, ms, pools):
                    self.gen = self._make(ms, pools)
                    self.done = False

                def _make(self, ms, pools):
                    cnt = 0
                    for m in ms:
                        o_sb = pool_osb.tile([128, D], BF, tag="osb",
                                             name=f"osb_{m}")
                        for cp in range(2):
                            pool, ptag = pools[cnt % len(pools)]
                            cnt += 1
                            tA = pool.tile([128, 512], F32, tag=ptag,
                                           name=f"op_{m}_{cp}_a")
                            tB = pool.tile([128, 512], F32, tag=ptag,
                                           name=f"op_{m}_{cp}_b")
                            for hi in range(4):
                                nc.tensor.matmul(
                                    tA[:], onorm[:, hi, ts(m, 128)],
                                    wo_sb[:, hi, ts(2 * cp, 512)],
                                    start=(hi == 0), stop=(hi == 3),
                                )
                                nc.tensor.matmul(
                                    tB[:], onorm[:, hi, ts(m, 128)],
                                    wo_sb[:, hi, ts(2 * cp + 1, 512)],
                                    start=(hi == 0), stop=(hi == 3),
                                )
                                if hi == 1:
                                    yield  # first half-unit (4 MMs)
                            nc.vector.tensor_copy(o_sb[:, ts(2 * cp, 512)], tA[:])
                            nc.scalar.copy(o_sb[:, ts(2 * cp + 1, 512)], tB[:])
                            nc.sync.dma_start(out[:, m, ts(cp, 1024)],
                                              o_sb[:, ts(cp, 1024)])
                            yield  # one ct-pair (8 MMs + 2 drains) issued

                def step(self, n=1):
                    if self.done:
                        return
                    try:
                        for _ in range(n):
                            next(self.gen)
                    except StopIteration:
                        self.done = True

                def flush(self):
                    self.step(10 ** 6)

            def attend_half(hi, half, filler=None):
                g = hi // 2
                st0 = 2 * half
                o_ps = [pp_o.tile([128, 512], F32, tag="o",
                                  name=f"ob_{hi}_{half}_{k}") for k in range(2)]
                sums = pool_sums.tile([128, 1024], BF, tag="sums",
                                      name=f"sum_{hi}_{half}")
                es = {}

                def qk(skt):
                    lg = pp_lg.tile([128, 1024], F32, tag="lg",
                                    name=f"lg_{hi}_{half}_{skt}")
                    for idx in range(2):
                        nc.tensor.matmul(
                            lg[:, ts(idx, 512)],
                            kr[:, g, ts(skt, 128)],
                            qr[:, hi, ts(st0 + idx, 512)],
                            start=True, stop=True,
                        )
                    e = pool_e.tile([128, 1024], BF, tag="exp",
                                    name=f"e_{hi}_{half}_{skt}")
                    nc.scalar.activation(
                        e[:], lg[:],
                        mybir.ActivationFunctionType.Exp,
                        scale=INV_SQRT_D,
                    )
                    es[skt] = e

                # software-pipelined: QK of skt+1 (and any woven PE work)
                # issues before PV of skt, so the PE never sits directly
                # behind the exp of the chunk it is about to consume.
                qk(0)
                for skt in range(NKC):
                    if skt + 1 < NKC:
                        qk(skt + 1)
                    if filler is not None and skt < 8:
                        filler.step(1)
                    e = es.pop(skt)
                    if skt == 0:
                        nc.vector.tensor_copy(sums[:], e[:])
                    else:
                        nc.vector.tensor_add(sums[:], sums[:], e[:])
                    for idx in range(2):
                        nc.tensor.matmul(
                            o_ps[idx][:],
                            vn[:, g, skt, :],
                            e[:, ts(idx, 512)],
                            start=(skt == 0),
                            stop=(skt == NKC - 1),
                        )
                # drain o immediately on DVE (frees PSUM for the next half)
                o_f = pool_of.tile([128, 1024], F32, tag="of",
                                   name=f"of_{hi}_{half}")
                for idx in range(2):
                    nc.vector.tensor_copy(o_f[:, ts(idx, 512)], o_ps[idx][:])
                # denominator: partition-reduce on PE, both st rows gathered
                # into one [1,1024] row, 1/d = exp(-ln(d)) in a single ACT
                # pair, broadcast on PE, normalize on DVE.
                drow = pool_db.tile([1, 1024], F32, tag="drow",
                                    name=f"drow_{hi}_{half}")
                for idx in range(2):
                    den = pp_rot.tile([128, 512], F32, tag="rot",
                                      name=f"den_{hi}_{half}_{idx}")
                    nc.tensor.matmul(den[0:1, :], ones_col[:],
                                     sums[:, ts(idx, 512)],
                                     start=True, stop=True)
                    nc.vector.tensor_copy(drow[:, ts(idx, 512)], den[0:1, :])
                lrow = pool_db.tile([1, 1024], F32, tag="lrow",
                                    name=f"lrow_{hi}_{half}")
                nc.scalar.activation(lrow[:], drow[:],
                                     mybir.ActivationFunctionType.Ln)
                rrow = pool_db.tile([1, 1024], BF, tag="rrow",
                                    name=f"rrow_{hi}_{half}")
                nc.scalar.activation(rrow[:], lrow[:],
                                     mybir.ActivationFunctionType.Exp,
                                     scale=-1.0)
                for idx in range(2):
                    bcp = pp_rot.tile([128, 512], F32, tag="rot",
                                      name=f"bcp_{hi}_{half}_{idx}")
                    nc.tensor.matmul(bcp[:], ones_row[:],
                                     rrow[:, ts(idx, 512)],
                                     start=True, stop=True)
                    nc.vector.tensor_mul(onorm[:, hi, ts(st0 + idx, 512)],
                                         o_f[:, ts(idx, 512)], bcp[:])

            for hi in range(4):
                attend_half(hi, 0)
            op_front = OutProj(range(8), [(pp_rot, "rot")])
            for hi in range(4):
                attend_half(hi, 1, filler=op_front)
            op_front.flush()
            op_tail = OutProj(range(8, 16), [(pp_rot, "rot"), (pp_o, "o")])
            op_tail.flush()

    _split_waits(nc)
    return nc


def _chunk128(arr):
    """(K*128, N) f32 -> [128, K, N] bf16 with [p, k, n] = arr[k*128+p, n]."""
    k = arr.shape[0] // 128
    return np.ascontiguousarray(
        arr.reshape(k, 128, arr.shape[1]).transpose(1, 0, 2)
    ).astype(NPBF)


def _rope_tables(dim):
    pos = np.arange(S, dtype=np.float32)
    inv = (10000.0 ** (-(np.arange(dim, dtype=np.float32)) / np.float32(dim))
           ).astype(np.float32)
    freqs = pos[:, None] * inv[None, :]
    return np.cos(freqs).astype(np.float32), np.sin(freqs).astype(np.float32)


def kernel(x, mask, Wq, Wk, Wv, Wo, bo):
    global _NC_CACHE
    assert np.asarray(mask).all(), "kernel specialized for all-true mask"
    x = np.asarray(x, dtype=np.float32)
    Wq = np.asarray(Wq, dtype=np.float32)
    Wk = np.asarray(Wk, dtype=np.float32)
    Wv = np.asarray(Wv, dtype=np.float32)
    Wo = np.asarray(Wo, dtype=np.float32)
    bo = np.asarray(bo, dtype=np.float32)

    cos_q, sin_q = _rope_tables(1024)
    cos_k, sin_k = _rope_tables(256)

    def blk(a, i):  # column block i (width 128) of a
        return a[:, i * 128:(i + 1) * 128]

    in_maps = []
    for c in range(8):
        b, j = c // 4, c % 4
        a0, a1 = 2 * j, 2 * j + 1
        g0 = 0 if j < 2 else 1

        xb = x[b]                                   # (S, D)
        xT3 = _chunk128(np.ascontiguousarray(xb.T))  # [128, 16, S]

        wq_sel = np.concatenate(
            [blk(Wq, a0), blk(Wq, a1), blk(Wq, a0 + 8), blk(Wq, a1 + 8)], axis=1)
        myblk = g0 + 2 * (j % 2)
        wk_sel = blk(Wk, myblk)
        wv_sel = blk(Wv, myblk)
        wo_sel = np.concatenate(
            [Wo[h * 128:(h + 1) * 128, :] for h in (a0, a1, a0 + 8, a1 + 8)],
            axis=0)

        cq_sel = _chunk128(np.ascontiguousarray(
            np.concatenate([blk(cos_q, a0), blk(cos_q, a1)], axis=1).T))
        sq_sel = _chunk128(np.ascontiguousarray(
            np.concatenate([blk(sin_q, a0), blk(sin_q, a1)], axis=1).T))
        ck_sel = np.ascontiguousarray(blk(cos_k, g0).T).astype(NPBF)
        sk_sel = np.ascontiguousarray(blk(sin_k, g0).T).astype(NPBF)

        in_maps.append({
            "xT": xT3,
            "wq": _chunk128(wq_sel),
            "wk": _chunk128(wk_sel),
            "wv": _chunk128(wv_sel),
            "wo": _chunk128(wo_sel),
            "cq": cq_sel, "sq": sq_sel, "ck": ck_sel, "sk": sk_sel,
        })

    global LAST_RESULT
    if _NC_CACHE is None:
        _NC_CACHE = _build_nc()
    res = run_bass_kernel_spmd(_NC_CACHE, in_maps, list(range(8)))
    LAST_RESULT = res

    partials = [
        res.results[c]["out"].astype(np.float32).transpose(1, 0, 2).reshape(S, D)
        for c in range(8)
    ]
    out = np.stack(
        [sum(partials[4 * b + j] for j in range(4)) for b in range(2)], axis=0
    )
    return (out + bo).astype(np.float32)


# revision 18
# speedup vs baseline: 3.1519x; 3.1519x over previous
"""GQA attention (b=2, s=2048, d=2048, H=16, Hkv=4, depth=128) on 8 trn2 cores.

Sharding: core c = 4*b + j (b in {0,1}, j in {0..3}) handles batch b and
q-heads {2j, 2j+1, 2j+8, 2j+9}.  This model's RoPE rotates the full projected
vector (pairing dim i with i + d/2), so roped q-head h mixes raw column
blocks {h mod 8, (h mod 8) + 8}; the head grouping above makes the Wq column
shard exactly 512 columns with no duplication.  Those q-heads attend kv-heads
{g0, g0+2} (g0 = 0 for j<2 else 1), which likewise pair up under RoPE.
Each core of a pair projects ONE raw k block and ONE v head; the pair swaps
them with a 2-way AllGather, halving the duplicated K/V projection work.
Wo is row-sharded over the 4 local head-dims; the 4 per-batch bf16 partials
are summed on the host (fp32) and bo added.

Device layout is fully transposed (feature dim on partitions): q_r^T, k_r^T
are [depth, s]; logits l^T = k_r^T.T @ q_r^T so softmax's reduce axis is on
partitions and PV needs no transposes.  All matmuls bf16 (fp32 PSUM).

Schedule (PE issue order == PE execution order):
  [KV proj][Q proj i=0,1][attends half-0 rounds h0..h3]
  [attends half-1 rounds h0..h3, out-proj for m 0..7 woven into the skt
   loops two matmuls at a time][out-proj tail m 8..15]
Attention softmax denominators: running bf16 sums on DVE (2x mode), one
ones-matmul partition-reduce + one broadcast matmul per (half, st) on PE,
reciprocal as exp(-ln(d)) on ACT over the broadcast tile.  o PSUM banks are
drained to SBUF f32 immediately (ACT) so the next half's PV can start.
"""
import numpy as np
import ml_dtypes
from contextlib import ExitStack

import concourse.bass as bass
import concourse.mybir as mybir
import concourse.tile as tile
from concourse.bass import ts
from concourse.bass_utils import run_bass_kernel_spmd

BF = mybir.dt.bfloat16
F32 = mybir.dt.float32
NPBF = ml_dtypes.bfloat16

S = 2048          # sequence length
D = 2048          # d_model
DEPTH = 128       # head dim
NKC = 16          # contraction chunks of 128 over d_model
NST = 4           # 512-wide s tiles
INV_SQRT_D = 1.0 / float(np.sqrt(np.float32(DEPTH)))

_NC_CACHE = None
LAST_RESULT = None  # BassKernelResults of the most recent run (for profiling)


def _split_waits(nc, limit=1):
    """walrus rejects instructions carrying more than a couple of sem waits
    ('Too many sync wait commands').  Move excess waits onto dedicated NoOps
    on the same engine, placed immediately before the instruction."""
    idx = 0
    for f in nc.m.functions:
        for blk in f.blocks:
            insts = blk.instructions
            out = []
            for inst in insts:
                si = inst.sync_info
                if si is not None and len(si.on_wait) > limit:
                    waits = list(si.on_wait)
                    extra, keep = waits[:-limit], waits[-limit:]
                    for w in extra:
                        nop = mybir.InstNoOp(name=f"waitsplit_{idx}", ins=[], outs=[])
                        idx += 1
                        nop.engine = inst.engine
                        nop.bass_nofuse = True
                        nop.sync_info = mybir.SyncInfo(on_wait=[w], on_update=[])
                        out.append(nop)
                    inst.sync_info = mybir.SyncInfo(
                        on_wait=keep, on_update=list(si.on_update)
                    )
                out.append(inst)
            insts[:] = out


def _build_nc():
    nc = bass.Bass(num_devices=8)
    xT = nc.dram_tensor("xT", [128, NKC, S], BF, kind="ExternalInput")
    wq = nc.dram_tensor("wq", [128, NKC, 512], BF, kind="ExternalInput")
    wk = nc.dram_tensor("wk", [128, NKC, 128], BF, kind="ExternalInput")
    wv = nc.dram_tensor("wv", [128, NKC, 128], BF, kind="ExternalInput")
    wo = nc.dram_tensor("wo", [128, 4, D], BF, kind="ExternalInput")
    cq = nc.dram_tensor("cq", [128, 2, S], BF, kind="ExternalInput")
    sq = nc.dram_tensor("sq", [128, 2, S], BF, kind="ExternalInput")
    ck = nc.dram_tensor("ck", [128, S], BF, kind="ExternalInput")
    sk = nc.dram_tensor("sk", [128, S], BF, kind="ExternalInput")
    out = nc.dram_tensor("out", [128, 16, D], BF, kind="ExternalOutput")

    with tile.TileContext(nc) as tc, ExitStack() as top:
        pool_p = top.enter_context(tc.tile_pool(name="persist", bufs=1))
        # PSUM: 4 banks logits (double-buffered [128,1024]) + 2 banks o
        # accumulators + 2 rotating banks (den/bcast/out-proj groups)
        pp_lg = top.enter_context(tc.tile_pool(name="pslg", bufs=2, space="PSUM"))
        pp_o = top.enter_context(tc.tile_pool(name="pso", bufs=2, space="PSUM"))
        pp_rot = top.enter_context(tc.tile_pool(name="psrot", bufs=2, space="PSUM"))

        qr = pool_p.tile([128, 4, S], BF)        # roped qT, slots [a0,a1,a0+8,a1+8]
        kr = pool_p.tile([128, 2, S], BF)        # roped kT,  slots [g0, g0+2]
        vn = pool_p.tile([128, 2, NKC, DEPTH], BF)  # v native [p, g, skc, dv]
        wo_sb = pool_p.tile([128, 4, D], BF)
        ones_col = pool_p.tile([128, 1], BF)
        ones_row = pool_p.tile([1, 128], BF)
        nc.vector.memset(ones_col[:], 1.0)
        nc.vector.memset(ones_row[:], 1.0)

        # ---------------- phase A: projections + rope -----------------
        with ExitStack() as pA:
            pool_x = pA.enter_context(tc.tile_pool(name="pax", bufs=16))
            pool_w = pA.enter_context(tc.tile_pool(name="paw", bufs=16))
            pool_tab = pA.enter_context(tc.tile_pool(name="pat", bufs=1))
            pool_raw = pA.enter_context(tc.tile_pool(name="parw", bufs=2))
            pool_kv = pA.enter_context(tc.tile_pool(name="pakv", bufs=1))
            pool_dram = pA.enter_context(tc.tile_pool(name="padr", bufs=1, space="DRAM"))

            # DMA issue queues: Sync gets x chunks + wq + tables (+ later the
            # collective returns and v transposes); Scalar gets wk/wv and the
            # kv-exchange send so the collective fires as early as possible.
            xTs, wqs, wks, wvs = [], [], [], []
            for kc in range(NKC):
                xt_t = pool_x.tile([128, S], BF, tag="xt", name=f"xt_{kc}")
                nc.sync.dma_start(xt_t[:], xT[:, kc, :])
                xTs.append(xt_t)
                wk_t = pool_w.tile([128, 128], BF, tag="wk", name=f"wk_{kc}")
                nc.scalar.dma_start(wk_t[:], wk[:, kc, :])
                wks.append(wk_t)
            for kc in range(NKC):
                wv_t = pool_w.tile([128, 128], BF, tag="wv", name=f"wv_{kc}")
                nc.scalar.dma_start(wv_t[:], wv[:, kc, :])
                wvs.append(wv_t)
            for kc in range(NKC):
                wq_t = pool_w.tile([128, 512], BF, tag="wq", name=f"wq_{kc}")
                nc.sync.dma_start(wq_t[:], wq[:, kc, :])
                wqs.append(wq_t)
            ck_sb = pool_tab.tile([128, S], BF)
            nc.sync.dma_start(ck_sb[:], ck[:])
            sk_sb = pool_tab.tile([128, S], BF)
            nc.sync.dma_start(sk_sb[:], sk[:])
            cq_sb = pool_tab.tile([128, 2, S], BF)
            sq_sb = pool_tab.tile([128, 2, S], BF)
            nc.sync.dma_start(cq_sb[:, 0, :], cq[:, 0, :])
            nc.sync.dma_start(sq_sb[:, 0, :], sq[:, 0, :])
            nc.sync.dma_start(cq_sb[:, 1, :], cq[:, 1, :])
            nc.sync.dma_start(sq_sb[:, 1, :], sq[:, 1, :])
            nc.sync.dma_start(wo_sb[:], wo[:])

            def proj_4st(w_of, drain_to):
                """contract over all 16 kc into 4 [128,512] st accumulators
                spread over all three PSUM pools (so consecutive calls
                double-buffer), then drain via ACT copies.
                w_of(kc) -> stationary AP; drain_to(st) -> SBUF AP."""
                acc_lg = pp_lg.tile([128, 1024], F32, tag="lg", name="acc_lg")
                acc_o = pp_o.tile([128, 512], F32, tag="o", name="acc_o")
                acc_r = pp_rot.tile([128, 512], F32, tag="rot", name="acc_r")
                accs = [acc_lg[:, 0:512], acc_lg[:, 512:1024], acc_o[:], acc_r[:]]
                for kc in range(NKC):
                    for st in range(NST):
                        nc.tensor.matmul(
                            accs[st],
                            w_of(kc),
                            xTs[kc][:, ts(st, 512)],
                            start=(kc == 0),
                            stop=(kc == NKC - 1),
                        )
                for st in range(NST):
                    nc.vector.tensor_copy(drain_to(st), accs[st])

            # K/V: each core of a pair projects ONE raw k block and ONE v
            # head; the pair exchanges them with an AllGather.
            kv_sb = pool_kv.tile([128, 2 * S], BF, tag="kvmine")
            proj_4st(lambda kc: wks[kc][:], lambda st: kv_sb[:, ts(st, 512)])
            proj_4st(lambda kc: wvs[kc][:], lambda st: kv_sb[:, ts(NST + st, 512)])
            kv_in = pool_dram.tile([128, 2 * S], BF)
            kv_out = pool_dram.tile([2, 128, 2 * S], BF)
            nc.scalar.dma_start(kv_in[:], kv_sb[:])
            nc.gpsimd.collective_compute(
                "AllGather",
                mybir.AluOpType.bypass,
                replica_groups=[[0, 1], [2, 3], [4, 5], [6, 7]],
                ins=[kv_in.opt()],
                outs=[kv_out.opt()],
            )
            kboth = pool_kv.tile([128, 2, S], BF, tag="kboth")
            vtboth = pool_kv.tile([128, 2, S], BF, tag="vtboth")
            for r in range(2):
                nc.sync.dma_start(kboth[:, r, :], kv_out[r, :, 0:S])
                nc.sync.dma_start(vtboth[:, r, :], kv_out[r, :, S:2 * S])
            # v native via DMA transpose (g=0 needed first)
            for g in range(2):
                for skt in range(NKC):
                    nc.sync.dma_start_transpose(
                        vn[:, g, skt, :], vtboth[:, g, ts(skt, 128)]
                    )

            def rope_pair(x1_of, x2_of, c_of, s_of, out1_of, out2_of):
                """roped = (x1*c - x2*s, x2*c + x1*s), [128,1024]-wide DVE ops."""
                for h2 in range(2):  # S in 1024 halves
                    sl = ts(h2, 1024)
                    x1, x2 = x1_of(sl), x2_of(sl)
                    c_ap, s_ap = c_of(sl), s_of(sl)
                    t1 = pool_raw.tile([128, 1024], BF, tag="rt1", bufs=1, name="t1")
                    t2 = pool_raw.tile([128, 1024], BF, tag="rt2", bufs=1, name="t2")
                    nc.vector.tensor_mul(t1[:], x1, c_ap)
                    nc.vector.tensor_mul(t2[:], x2, s_ap)
                    nc.vector.tensor_sub(out1_of(sl), t1[:], t2[:])
                    t3 = pool_raw.tile([128, 1024], BF, tag="rt1", bufs=1, name="t3")
                    t4 = pool_raw.tile([128, 1024], BF, tag="rt2", bufs=1, name="t4")
                    nc.vector.tensor_mul(t3[:], x2, c_ap)
                    nc.vector.tensor_mul(t4[:], x1, s_ap)
                    nc.vector.tensor_add(out2_of(sl), t3[:], t4[:])

            # Q raw pair (i, 2+i) -> roped qr slots (i, 2+i).  The k rope is
            # emitted between i=0 and i=1 so the DVE FIFO doesn't block the
            # early q ropes on the collective's arrival.
            for i in range(2):
                raw1 = pool_raw.tile([128, S], BF, tag="raw1", bufs=2, name=f"q1_{i}")
                raw2 = pool_raw.tile([128, S], BF, tag="raw2", bufs=2, name=f"q2_{i}")
                proj_4st(lambda kc: wqs[kc][:, ts(i, 128)],
                         lambda st: raw1[:, ts(st, 512)])
                proj_4st(lambda kc: wqs[kc][:, ts(2 + i, 128)],
                         lambda st: raw2[:, ts(st, 512)])
                rope_pair(lambda sl: raw1[:, sl], lambda sl: raw2[:, sl],
                          lambda sl: cq_sb[:, i, sl], lambda sl: sq_sb[:, i, sl],
                          lambda sl: qr[:, i, sl], lambda sl: qr[:, 2 + i, sl])
                if i == 0:
                    rope_pair(lambda sl: kboth[:, 0, sl],
                              lambda sl: kboth[:, 1, sl],
                              lambda sl: ck_sb[:, sl], lambda sl: sk_sb[:, sl],
                              lambda sl: kr[:, 0, sl], lambda sl: kr[:, 1, sl])

        # ------------- phase B: attention + output projection -------------
        with ExitStack() as pB:
            pool_e = pB.enter_context(tc.tile_pool(name="pe", bufs=8))
            pool_sums = pB.enter_context(tc.tile_pool(name="psum_s", bufs=2))
            pool_of = pB.enter_context(tc.tile_pool(name="pof", bufs=2))
            pool_db = pB.enter_context(tc.tile_pool(name="pdb", bufs=4))
            pool_osb = pB.enter_context(tc.tile_pool(name="posb", bufs=3))
            pool_on = pB.enter_context(tc.tile_pool(name="pon", bufs=1))
            onorm = pool_on.tile([128, 4, S], BF)  # normalized o^T per head

            class OutProj:
                """Emits the output projection for m-blocks (128 rows of sq)
                one ct-pair at a time (4 hi x 2 ct accumulating matmuls into
                2 rotating PSUM banks + drains) so it can be woven into
                attend skt loops."""
                def __init__(self, ms, pools):
                    self.gen = self._make(ms, pools)
                    self.done = False

                def _make(self, ms, pools):
                    cnt = 0
                    for m in ms:
                        o_sb = pool_osb.tile([128, D], BF, tag="osb",
                                             name=f"osb_{m}")
                        for cp in range(2):
                            pool, ptag = pools[cnt % len(pools)]
                            cnt += 1
                            tA = pool.tile([128, 512], F32, tag=ptag,
                                           name=f"op_{m}_{cp}_a")
                            tB = pool.tile([128, 512], F32, tag=ptag,
                                           name=f"op_{m}_{cp}_b")
                            for hi in range(4):
                                nc.tensor.matmul(
                                    tA[:], onorm[:, hi, ts(m, 128)],
                                    wo_sb[:, hi, ts(2 * cp, 512)],
                                    start=(hi == 0), stop=(hi == 3),
                                )
                                nc.tensor.matmul(
                                    tB[:], onorm[:, hi, ts(m, 128)],
                                    wo_sb[:, hi, ts(2 * cp + 1, 512)],
                                    start=(hi == 0), stop=(hi == 3),
                                )
                            nc.vector.tensor_copy(o_sb[:, ts(2 * cp, 512)], tA[:])
                            nc.scalar.copy(o_sb[:, ts(2 * cp + 1, 512)], tB[:])
                            nc.sync.dma_start(out[:, m, ts(cp, 1024)],
                                              o_sb[:, ts(cp, 1024)])
                            yield  # one ct-pair (8 MMs + 2 drains) issued

                def step(self, n=1):
                    if self.done:
                        return
                    try:
                        for _ in range(n):
                            next(self.gen)
                    except StopIteration:
                        self.done = True

                def flush(self):
                    self.step(10 ** 6)

            def attend_half(hi, half, filler=None):
                g = hi // 2
                st0 = 2 * half
                o_ps = [pp_o.tile([128, 512], F32, tag="o",
                                  name=f"ob_{hi}_{half}_{k}") for k in range(2)]
                sums = pool_sums.tile([128, 1024], BF, tag="sums",
                                      name=f"sum_{hi}_{half}")
                es = {}

                def qk(skt):
                    lg = pp_lg.tile([128, 1024], F32, tag="lg",
                                    name=f"lg_{hi}_{half}_{skt}")
                    for idx in range(2):
                        nc.tensor.matmul(
                            lg[:, ts(idx, 512)],
                            kr[:, g, ts(skt, 128)],
                            qr[:, hi, ts(st0 + idx, 512)],
                            start=True, stop=True,
                        )
                    e = pool_e.tile([128, 1024], BF, tag="exp",
                                    name=f"e_{hi}_{half}_{skt}")
                    nc.scalar.activation(
                        e[:], lg[:],
                        mybir.ActivationFunctionType.Exp,
                        scale=INV_SQRT_D,
                    )
                    es[skt] = e

                # software-pipelined: QK of skt+1 (and any woven PE work)
                # issues before PV of skt, so the PE never sits directly
                # behind the exp of the chunk it is about to consume.
                qk(0)
                for skt in range(NKC):
                    if skt + 1 < NKC:
                        qk(skt + 1)
                    if filler is not None and skt in (0, 2, 4, 6, 8):
                        filler.step(1)
                    e = es.pop(skt)
                    if skt == 0:
                        nc.vector.tensor_copy(sums[:], e[:])
                    else:
                        nc.vector.tensor_add(sums[:], sums[:], e[:])
                    for idx in range(2):
                        nc.tensor.matmul(
                            o_ps[idx][:],
                            vn[:, g, skt, :],
                            e[:, ts(idx, 512)],
                            start=(skt == 0),
                            stop=(skt == NKC - 1),
                        )
                # drain o immediately on DVE (frees PSUM for the next half)
                o_f = pool_of.tile([128, 1024], F32, tag="of",
                                   name=f"of_{hi}_{half}")
                for idx in range(2):
                    nc.vector.tensor_copy(o_f[:, ts(idx, 512)], o_ps[idx][:])
                # denominator: partition-reduce on PE, both st rows gathered
                # into one [1,1024] row, 1/d = exp(-ln(d)) in a single ACT
                # pair, broadcast on PE, normalize on DVE.
                drow = pool_db.tile([1, 1024], F32, tag="drow",
                                    name=f"drow_{hi}_{half}")
                for idx in range(2):
                    den = pp_rot.tile([128, 512], F32, tag="rot",
                                      name=f"den_{hi}_{half}_{idx}")
                    nc.tensor.matmul(den[0:1, :], ones_col[:],
                                     sums[:, ts(idx, 512)],
                                     start=True, stop=True)
                    nc.vector.tensor_copy(drow[:, ts(idx, 512)], den[0:1, :])
                lrow = pool_db.tile([1, 1024], F32, tag="lrow",
                                    name=f"lrow_{hi}_{half}")
                nc.scalar.activation(lrow[:], drow[:],
                                     mybir.ActivationFunctionType.Ln)
                rrow = pool_db.tile([1, 1024], BF, tag="rrow",
                                    name=f"rrow_{hi}_{half}")
                nc.scalar.activation(rrow[:], lrow[:],
                                     mybir.ActivationFunctionType.Exp,
                                     scale=-1.0)
                for idx in range(2):
                    bcp = pp_rot.tile([128, 512], F32, tag="rot",
                                      name=f"bcp_{hi}_{half}_{idx}")
                    nc.tensor.matmul(bcp[:], ones_row[:],
                                     rrow[:, ts(idx, 512)],
                                     start=True, stop=True)
                    nc.vector.tensor_mul(onorm[:, hi, ts(st0 + idx, 512)],
                                         o_f[:, ts(idx, 512)], bcp[:])

            for hi in range(4):
                attend_half(hi, 0)
            op_front = OutProj(range(8), [(pp_rot, "rot")])
            for hi in range(4):
                attend_half(hi, 1, filler=op_front)
            op_front.flush()
            op_tail = OutProj(range(8, 16), [(pp_rot, "rot"), (pp_o, "o")])
            op_tail.flush()

    _split_waits(nc)
    return nc


def _chunk128(arr):
    """(K*128, N) f32 -> [128, K, N] bf16 with [p, k, n] = arr[k*128+p, n]."""
    k = arr.shape[0] // 128
    return np.ascontiguousarray(
        arr.reshape(k, 128, arr.shape[1]).transpose(1, 0, 2)
    ).astype(NPBF)


def _rope_tables(dim):
    pos = np.arange(S, dtype=np.float32)
    inv = (10000.0 ** (-(np.arange(dim, dtype=np.float32)) / np.float32(dim))
           ).astype(np.float32)
    freqs = pos[:, None] * inv[None, :]
    return np.cos(freqs).astype(np.float32), np.sin(freqs).astype(np.float32)


def kernel(x, mask, Wq, Wk, Wv, Wo, bo):
    global _NC_CACHE
    assert np.asarray(mask).all(), "kernel specialized for all-true mask"
    x = np.asarray(x, dtype=np.float32)
    Wq = np.asarray(Wq, dtype=np.float32)
    Wk = np.asarray(Wk, dtype=np.float32)
    Wv = np.asarray(Wv, dtype=np.float32)
    Wo = np.asarray(Wo, dtype=np.float32)
    bo = np.asarray(bo, dtype=np.float32)

    cos_q, sin_q = _rope_tables(1024)
    cos_k, sin_k = _rope_tables(256)

    def blk(a, i):  # column block i (width 128) of a
        return a[:, i * 128:(i + 1) * 128]

    in_maps = []
    for c in range(8):
        b, j = c // 4, c % 4
        a0, a1 = 2 * j, 2 * j + 1
        g0 = 0 if j < 2 else 1

        xb = x[b]                                   # (S, D)
        xT3 = _chunk128(np.ascontiguousarray(xb.T))  # [128, 16, S]

        wq_sel = np.concatenate(
            [blk(Wq, a0), blk(Wq, a1), blk(Wq, a0 + 8), blk(Wq, a1 + 8)], axis=1)
        myblk = g0 + 2 * (j % 2)
        wk_sel = blk(Wk, myblk)
        wv_sel = blk(Wv, myblk)
        wo_sel = np.concatenate(
            [Wo[h * 128:(h + 1) * 128, :] for h in (a0, a1, a0 + 8, a1 + 8)],
            axis=0)

        cq_sel = _chunk128(np.ascontiguousarray(
            np.concatenate([blk(cos_q, a0), blk(cos_q, a1)], axis=1).T))
        sq_sel = _chunk128(np.ascontiguousarray(
            np.concatenate([blk(sin_q, a0), blk(sin_q, a1)], axis=1).T))
        ck_sel = np.ascontiguousarray(blk(cos_k, g0).T).astype(NPBF)
        sk_sel = np.ascontiguousarray(blk(sin_k, g0).T).astype(NPBF)

        in_maps.append({
            "xT": xT3,
            "wq": _chunk128(wq_sel),
            "wk": _chunk128(wk_sel),
            "wv": _chunk128(wv_sel),
            "wo": _chunk128(wo_sel),
            "cq": cq_sel, "sq": sq_sel, "ck": ck_sel, "sk": sk_sel,
        })

    global LAST_RESULT
    if _NC_CACHE is None:
        _NC_CACHE = _build_nc()
    res = run_bass_kernel_spmd(_NC_CACHE, in_maps, list(range(8)))
    LAST_RESULT = res

    partials = [
        res.results[c]["out"].astype(np.float32).transpose(1, 0, 2).reshape(S, D)
        for c in range(8)
    ]
    out = np.stack(
        [sum(partials[4 * b + j] for j in range(4)) for b in range(2)], axis=0
    )
    return (out + bo).astype(np.float32)


# revision 19
# speedup vs baseline: 3.1608x; 1.0028x over previous
"""GQA attention (b=2, s=2048, d=2048, H=16, Hkv=4, depth=128) on 8 trn2 cores.

Sharding: core c = 4*b + j (b in {0,1}, j in {0..3}) handles batch b and
q-heads {2j, 2j+1, 2j+8, 2j+9}.  This model's RoPE rotates the full projected
vector (pairing dim i with i + d/2), so roped q-head h mixes raw column
blocks {h mod 8, (h mod 8) + 8}; the head grouping above makes the Wq column
shard exactly 512 columns with no duplication.  Those q-heads attend kv-heads
{g0, g0+2} (g0 = 0 for j<2 else 1), which likewise pair up under RoPE.
Each core of a pair projects ONE raw k block and ONE v head; the pair swaps
them with a 2-way AllGather, halving the duplicated K/V projection work.
Wo is row-sharded over the 4 local head-dims; the 4 per-batch bf16 partials
are summed on the host (fp32) and bo added.

Device layout is fully transposed (feature dim on partitions): q_r^T, k_r^T
are [depth, s]; logits l^T = k_r^T.T @ q_r^T so softmax's reduce axis is on
partitions and PV needs no transposes.  All matmuls bf16 (fp32 PSUM).

Schedule (PE issue order == PE execution order):
  [KV proj][Q proj i=0,1][attends half-0 rounds h0..h3]
  [attends half-1 rounds h0..h3, out-proj for m 0..7 woven into the skt
   loops two matmuls at a time][out-proj tail m 8..15]
Attention softmax denominators: running bf16 sums on DVE (2x mode), one
ones-matmul partition-reduce + one broadcast matmul per (half, st) on PE,
reciprocal as one exp(-ln(d)) pair per half on the merged [1,1024] row (ACT).
o PSUM banks are drained to SBUF f32 immediately (DVE) so the next half's PV
can start; the attend skt loop is software-pipelined (QK of skt+1 and woven
out-proj matmuls issue before PV of skt).
"""
import numpy as np
import ml_dtypes
from contextlib import ExitStack

import concourse.bass as bass
import concourse.mybir as mybir
import concourse.tile as tile
from concourse.bass import ts
from concourse.bass_utils import run_bass_kernel_spmd

BF = mybir.dt.bfloat16
F32 = mybir.dt.float32
NPBF = ml_dtypes.bfloat16

S = 2048          # sequence length
D = 2048          # d_model
DEPTH = 128       # head dim
NKC = 16          # contraction chunks of 128 over d_model
NST = 4           # 512-wide s tiles
INV_SQRT_D = 1.0 / float(np.sqrt(np.float32(DEPTH)))

_NC_CACHE = None
LAST_RESULT = None  # BassKernelResults of the most recent run (for profiling)


def _split_waits(nc, limit=1):
    """walrus rejects instructions carrying more than a couple of sem waits
    ('Too many sync wait commands').  Move excess waits onto dedicated NoOps
    on the same engine, placed immediately before the instruction."""
    idx = 0
    for f in nc.m.functions:
        for blk in f.blocks:
            insts = blk.instructions
            out = []
            for inst in insts:
                si = inst.sync_info
                if si is not None and len(si.on_wait) > limit:
                    waits = list(si.on_wait)
                    extra, keep = waits[:-limit], waits[-limit:]
                    for w in extra:
                        nop = mybir.InstNoOp(name=f"waitsplit_{idx}", ins=[], outs=[])
                        idx += 1
                        nop.engine = inst.engine
                        nop.bass_nofuse = True
                        nop.sync_info = mybir.SyncInfo(on_wait=[w], on_update=[])
                        out.append(nop)
                    inst.sync_info = mybir.SyncInfo(
                        on_wait=keep, on_update=list(si.on_update)
                    )
                out.append(inst)
            insts[:] = out


def _build_nc():
    nc = bass.Bass(num_devices=8)
    xT = nc.dram_tensor("xT", [128, NKC, S], BF, kind="ExternalInput")
    wq = nc.dram_tensor("wq", [128, NKC, 512], BF, kind="ExternalInput")
    wk = nc.dram_tensor("wk", [128, NKC, 128], BF, kind="ExternalInput")
    wv = nc.dram_tensor("wv", [128, NKC, 128], BF, kind="ExternalInput")
    wo = nc.dram_tensor("wo", [128, 4, D], BF, kind="ExternalInput")
    cq = nc.dram_tensor("cq", [128, 2, S], BF, kind="ExternalInput")
    sq = nc.dram_tensor("sq", [128, 2, S], BF, kind="ExternalInput")
    ck = nc.dram_tensor("ck", [128, S], BF, kind="ExternalInput")
    sk = nc.dram_tensor("sk", [128, S], BF, kind="ExternalInput")
    out = nc.dram_tensor("out", [128, 16, D], BF, kind="ExternalOutput")

    with tile.TileContext(nc) as tc, ExitStack() as top:
        pool_p = top.enter_context(tc.tile_pool(name="persist", bufs=1))
        # PSUM: 4 banks logits (double-buffered [128,1024]) + 2 banks o
        # accumulators + 2 rotating banks (den/bcast/out-proj groups)
        pp_lg = top.enter_context(tc.tile_pool(name="pslg", bufs=2, space="PSUM"))
        pp_o = top.enter_context(tc.tile_pool(name="pso", bufs=2, space="PSUM"))
        pp_rot = top.enter_context(tc.tile_pool(name="psrot", bufs=2, space="PSUM"))

        qr = pool_p.tile([128, 4, S], BF)        # roped qT, slots [a0,a1,a0+8,a1+8]
        kr = pool_p.tile([128, 2, S], BF)        # roped kT,  slots [g0, g0+2]
        vn = pool_p.tile([128, 2, NKC, DEPTH], BF)  # v native [p, g, skc, dv]
        wo_sb = pool_p.tile([128, 4, D], BF)
        ones_col = pool_p.tile([128, 1], BF)
        ones_row = pool_p.tile([1, 128], BF)
        nc.vector.memset(ones_col[:], 1.0)
        nc.vector.memset(ones_row[:], 1.0)

        # ---------------- phase A: projections + rope -----------------
        with ExitStack() as pA:
            pool_x = pA.enter_context(tc.tile_pool(name="pax", bufs=16))
            pool_w = pA.enter_context(tc.tile_pool(name="paw", bufs=16))
            pool_tab = pA.enter_context(tc.tile_pool(name="pat", bufs=1))
            pool_raw = pA.enter_context(tc.tile_pool(name="parw", bufs=2))
            pool_kv = pA.enter_context(tc.tile_pool(name="pakv", bufs=1))
            pool_dram = pA.enter_context(tc.tile_pool(name="padr", bufs=1, space="DRAM"))

            # DMA issue queues: Sync gets x chunks + wq + tables (+ later the
            # collective returns and v transposes); Scalar gets wk/wv and the
            # kv-exchange send so the collective fires as early as possible.
            xTs, wqs, wks, wvs = [], [], [], []
            for kc in range(NKC):
                xt_t = pool_x.tile([128, S], BF, tag="xt", name=f"xt_{kc}")
                nc.sync.dma_start(xt_t[:], xT[:, kc, :])
                xTs.append(xt_t)
                wk_t = pool_w.tile([128, 128], BF, tag="wk", name=f"wk_{kc}")
                nc.scalar.dma_start(wk_t[:], wk[:, kc, :])
                wks.append(wk_t)
            for kc in range(NKC):
                wv_t = pool_w.tile([128, 128], BF, tag="wv", name=f"wv_{kc}")
                nc.scalar.dma_start(wv_t[:], wv[:, kc, :])
                wvs.append(wv_t)
            for kc in range(NKC):
                wq_t = pool_w.tile([128, 512], BF, tag="wq", name=f"wq_{kc}")
                nc.sync.dma_start(wq_t[:], wq[:, kc, :])
                wqs.append(wq_t)
            ck_sb = pool_tab.tile([128, S], BF)
            nc.sync.dma_start(ck_sb[:], ck[:])
            sk_sb = pool_tab.tile([128, S], BF)
            nc.sync.dma_start(sk_sb[:], sk[:])
            cq_sb = pool_tab.tile([128, 2, S], BF)
            sq_sb = pool_tab.tile([128, 2, S], BF)
            nc.sync.dma_start(cq_sb[:, 0, :], cq[:, 0, :])
            nc.sync.dma_start(sq_sb[:, 0, :], sq[:, 0, :])
            nc.sync.dma_start(cq_sb[:, 1, :], cq[:, 1, :])
            nc.sync.dma_start(sq_sb[:, 1, :], sq[:, 1, :])
            nc.sync.dma_start(wo_sb[:], wo[:])

            def proj_4st(w_of, drain_to):
                """contract over all 16 kc into 4 [128,512] st accumulators
                spread over all three PSUM pools (so consecutive calls
                double-buffer), then drain via ACT copies.
                w_of(kc) -> stationary AP; drain_to(st) -> SBUF AP."""
                acc_lg = pp_lg.tile([128, 1024], F32, tag="lg", name="acc_lg")
                acc_o = pp_o.tile([128, 512], F32, tag="o", name="acc_o")
                acc_r = pp_rot.tile([128, 512], F32, tag="rot", name="acc_r")
                accs = [acc_lg[:, 0:512], acc_lg[:, 512:1024], acc_o[:], acc_r[:]]
                for kc in range(NKC):
                    for st in range(NST):
                        nc.tensor.matmul(
                            accs[st],
                            w_of(kc),
                            xTs[kc][:, ts(st, 512)],
                            start=(kc == 0),
                            stop=(kc == NKC - 1),
                        )
                for st in range(NST):
                    nc.vector.tensor_copy(drain_to(st), accs[st])

            # K/V: each core of a pair projects ONE raw k block and ONE v
            # head; the pair exchanges them with an AllGather.
            kv_sb = pool_kv.tile([128, 2 * S], BF, tag="kvmine")
            proj_4st(lambda kc: wks[kc][:], lambda st: kv_sb[:, ts(st, 512)])
            proj_4st(lambda kc: wvs[kc][:], lambda st: kv_sb[:, ts(NST + st, 512)])
            kv_in = pool_dram.tile([128, 2 * S], BF)
            kv_out = pool_dram.tile([2, 128, 2 * S], BF)
            nc.scalar.dma_start(kv_in[:], kv_sb[:])
            nc.gpsimd.collective_compute(
                "AllGather",
                mybir.AluOpType.bypass,
                replica_groups=[[0, 1], [2, 3], [4, 5], [6, 7]],
                ins=[kv_in.opt()],
                outs=[kv_out.opt()],
            )
            kboth = pool_kv.tile([128, 2, S], BF, tag="kboth")
            vtboth = pool_kv.tile([128, 2, S], BF, tag="vtboth")
            for r in range(2):
                nc.sync.dma_start(kboth[:, r, :], kv_out[r, :, 0:S])
                nc.sync.dma_start(vtboth[:, r, :], kv_out[r, :, S:2 * S])
            # v native via DMA transpose (g=0 needed first)
            for g in range(2):
                for skt in range(NKC):
                    nc.sync.dma_start_transpose(
                        vn[:, g, skt, :], vtboth[:, g, ts(skt, 128)]
                    )

            def rope_pair(x1_of, x2_of, c_of, s_of, out1_of, out2_of):
                """roped = (x1*c - x2*s, x2*c + x1*s), [128,1024]-wide DVE ops."""
                for h2 in range(2):  # S in 1024 halves
                    sl = ts(h2, 1024)
                    x1, x2 = x1_of(sl), x2_of(sl)
                    c_ap, s_ap = c_of(sl), s_of(sl)
                    t1 = pool_raw.tile([128, 1024], BF, tag="rt1", bufs=1, name="t1")
                    t2 = pool_raw.tile([128, 1024], BF, tag="rt2", bufs=1, name="t2")
                    nc.vector.tensor_mul(t1[:], x1, c_ap)
                    nc.vector.tensor_mul(t2[:], x2, s_ap)
                    nc.vector.tensor_sub(out1_of(sl), t1[:], t2[:])
                    t3 = pool_raw.tile([128, 1024], BF, tag="rt1", bufs=1, name="t3")
                    t4 = pool_raw.tile([128, 1024], BF, tag="rt2", bufs=1, name="t4")
                    nc.vector.tensor_mul(t3[:], x2, c_ap)
                    nc.vector.tensor_mul(t4[:], x1, s_ap)
                    nc.vector.tensor_add(out2_of(sl), t3[:], t4[:])

            # Q raw pair (i, 2+i) -> roped qr slots (i, 2+i).  The k rope is
            # emitted between i=0 and i=1 so the DVE FIFO doesn't block the
            # early q ropes on the collective's arrival.
            for i in range(2):
                raw1 = pool_raw.tile([128, S], BF, tag="raw1", bufs=2, name=f"q1_{i}")
                raw2 = pool_raw.tile([128, S], BF, tag="raw2", bufs=2, name=f"q2_{i}")
                proj_4st(lambda kc: wqs[kc][:, ts(i, 128)],
                         lambda st: raw1[:, ts(st, 512)])
                proj_4st(lambda kc: wqs[kc][:, ts(2 + i, 128)],
                         lambda st: raw2[:, ts(st, 512)])
                rope_pair(lambda sl: raw1[:, sl], lambda sl: raw2[:, sl],
                          lambda sl: cq_sb[:, i, sl], lambda sl: sq_sb[:, i, sl],
                          lambda sl: qr[:, i, sl], lambda sl: qr[:, 2 + i, sl])
                if i == 0:
                    rope_pair(lambda sl: kboth[:, 0, sl],
                              lambda sl: kboth[:, 1, sl],
                              lambda sl: ck_sb[:, sl], lambda sl: sk_sb[:, sl],
                              lambda sl: kr[:, 0, sl], lambda sl: kr[:, 1, sl])

        # ------------- phase B: attention + output projection -------------
        with ExitStack() as pB:
            pool_e = pB.enter_context(tc.tile_pool(name="pe", bufs=8))
            pool_sums = pB.enter_context(tc.tile_pool(name="psum_s", bufs=2))
            pool_of = pB.enter_context(tc.tile_pool(name="pof", bufs=2))
            pool_db = pB.enter_context(tc.tile_pool(name="pdb", bufs=4))
            pool_osb = pB.enter_context(tc.tile_pool(name="posb", bufs=3))
            pool_on = pB.enter_context(tc.tile_pool(name="pon", bufs=1))
            onorm = pool_on.tile([128, 4, S], BF)  # normalized o^T per head

            class OutProj:
                """Emits the output projection for m-blocks (128 rows of sq)
                one ct-pair at a time (4 hi x 2 ct accumulating matmuls into
                2 rotating PSUM banks + drains) so it can be woven into
                attend skt loops."""
                def __init__(self, ms, pools):
                    self.gen = self._make(ms, pools)
                    self.done = False

                def _make(self, ms, pools):
                    cnt = 0
                    for m in ms:
                        o_sb = pool_osb.tile([128, D], BF, tag="osb",
                                             name=f"osb_{m}")
                        for cp in range(2):
                            pool, ptag = pools[cnt % len(pools)]
                            cnt += 1
                            tA = pool.tile([128, 512], F32, tag=ptag,
                                           name=f"op_{m}_{cp}_a")
                            tB = pool.tile([128, 512], F32, tag=ptag,
                                           name=f"op_{m}_{cp}_b")
                            for hi in range(4):
                                nc.tensor.matmul(
                                    tA[:], onorm[:, hi, ts(m, 128)],
                                    wo_sb[:, hi, ts(2 * cp, 512)],
                                    start=(hi == 0), stop=(hi == 3),
                                )
                                nc.tensor.matmul(
                                    tB[:], onorm[:, hi, ts(m, 128)],
                                    wo_sb[:, hi, ts(2 * cp + 1, 512)],
                                    start=(hi == 0), stop=(hi == 3),
                                )
                            nc.vector.tensor_copy(o_sb[:, ts(2 * cp, 512)], tA[:])
                            nc.scalar.copy(o_sb[:, ts(2 * cp + 1, 512)], tB[:])
                            nc.sync.dma_start(out[:, m, ts(cp, 1024)],
                                              o_sb[:, ts(cp, 1024)])
                            yield  # one ct-pair (8 MMs + 2 drains) issued

                def step(self, n=1):
                    if self.done:
                        return
                    try:
                        for _ in range(n):
                            next(self.gen)
                    except StopIteration:
                        self.done = True

                def flush(self):
                    self.step(10 ** 6)

            def attend_half(hi, half, filler=None):
                g = hi // 2
                st0 = 2 * half
                o_ps = [pp_o.tile([128, 512], F32, tag="o",
                                  name=f"ob_{hi}_{half}_{k}") for k in range(2)]
                sums = pool_sums.tile([128, 1024], BF, tag="sums",
                                      name=f"sum_{hi}_{half}")
                es = {}

                def qk(skt):
                    lg = pp_lg.tile([128, 1024], F32, tag="lg",
                                    name=f"lg_{hi}_{half}_{skt}")
                    for idx in range(2):
                        nc.tensor.matmul(
                            lg[:, ts(idx, 512)],
                            kr[:, g, ts(skt, 128)],
                            qr[:, hi, ts(st0 + idx, 512)],
                            start=True, stop=True,
                        )
                    e = pool_e.tile([128, 1024], BF, tag="exp",
                                    name=f"e_{hi}_{half}_{skt}")
                    nc.scalar.activation(
                        e[:], lg[:],
                        mybir.ActivationFunctionType.Exp,
                        scale=INV_SQRT_D,
                    )
                    es[skt] = e

                # software-pipelined: QK of skt+1 (and any woven PE work)
                # issues before PV of skt, so the PE never sits directly
                # behind the exp of the chunk it is about to consume.
                qk(0)
                for skt in range(NKC):
                    if skt + 1 < NKC:
                        qk(skt + 1)
                    if filler is not None and skt in (0, 2, 4, 6, 8):
                        filler.step(1)
                    e = es.pop(skt)
                    if skt == 0:
                        nc.vector.tensor_copy(sums[:], e[:])
                    else:
                        nc.vector.tensor_add(sums[:], sums[:], e[:])
                    for idx in range(2):
                        nc.tensor.matmul(
                            o_ps[idx][:],
                            vn[:, g, skt, :],
                            e[:, ts(idx, 512)],
                            start=(skt == 0),
                            stop=(skt == NKC - 1),
                        )
                # drain o immediately on DVE (frees PSUM for the next half)
                o_f = pool_of.tile([128, 1024], F32, tag="of",
                                   name=f"of_{hi}_{half}")
                for idx in range(2):
                    nc.vector.tensor_copy(o_f[:, ts(idx, 512)], o_ps[idx][:])
                # denominator: partition-reduce on PE, both st rows gathered
                # into one [1,1024] row, 1/d = exp(-ln(d)) in a single ACT
                # pair, broadcast on PE, normalize on DVE.
                drow = pool_db.tile([1, 1024], F32, tag="drow",
                                    name=f"drow_{hi}_{half}")
                for idx in range(2):
                    den = pp_rot.tile([128, 512], F32, tag="rot",
                                      name=f"den_{hi}_{half}_{idx}")
                    nc.tensor.matmul(den[0:1, :], ones_col[:],
                                     sums[:, ts(idx, 512)],
                                     start=True, stop=True)
                    nc.vector.tensor_copy(drow[:, ts(idx, 512)], den[0:1, :])
                lrow = pool_db.tile([1, 1024], F32, tag="lrow",
                                    name=f"lrow_{hi}_{half}")
                nc.scalar.activation(lrow[:], drow[:],
                                     mybir.ActivationFunctionType.Ln)
                rrow = pool_db.tile([1, 1024], BF, tag="rrow",
                                    name=f"rrow_{hi}_{half}")
                nc.scalar.activation(rrow[:], lrow[:],
                                     mybir.ActivationFunctionType.Exp,
                                     scale=-1.0)
                for idx in range(2):
                    bcp = pp_rot.tile([128, 512], F32, tag="rot",
                                      name=f"bcp_{hi}_{half}_{idx}")
                    nc.tensor.matmul(bcp[:], ones_row[:],
                                     rrow[:, ts(idx, 512)],
                                     start=True, stop=True)
                    nc.vector.tensor_mul(onorm[:, hi, ts(st0 + idx, 512)],
                                         o_f[:, ts(idx, 512)], bcp[:])

            for hi in range(4):
                attend_half(hi, 0)
            op_front = OutProj(range(8), [(pp_rot, "rot")])
            for hi in range(4):
                attend_half(hi, 1, filler=op_front)
            op_front.flush()
            op_tail = OutProj(range(8, 16), [(pp_rot, "rot"), (pp_o, "o")])
            op_tail.flush()

    _split_waits(nc)
    return nc


def _chunk128(arr):
    """(K*128, N) f32 -> [128, K, N] bf16 with [p, k, n] = arr[k*128+p, n]."""
    k = arr.shape[0] // 128
    return np.ascontiguousarray(
        arr.reshape(k, 128, arr.shape[1]).transpose(1, 0, 2)
    ).astype(NPBF)


def _rope_tables(dim):
    pos = np.arange(S, dtype=np.float32)
    inv = (10000.0 ** (-(np.arange(dim, dtype=np.float32)) / np.float32(dim))
           ).astype(np.float32)
    freqs = pos[:, None] * inv[None, :]
    return np.cos(freqs).astype(np.float32), np.sin(freqs).astype(np.float32)


def kernel(x, mask, Wq, Wk, Wv, Wo, bo):
    global _NC_CACHE
    assert np.asarray(mask).all(), "kernel specialized for all-true mask"
    x = np.asarray(x, dtype=np.float32)
    Wq = np.asarray(Wq, dtype=np.float32)
    Wk = np.asarray(Wk, dtype=np.float32)
    Wv = np.asarray(Wv, dtype=np.float32)
    Wo = np.asarray(Wo, dtype=np.float32)
    bo = np.asarray(bo, dtype=np.float32)

    cos_q, sin_q = _rope_tables(1024)
    cos_k, sin_k = _rope_tables(256)

    def blk(a, i):  # column block i (width 128) of a
        return a[:, i * 128:(i + 1) * 128]

    in_maps = []
    for c in range(8):
        b, j = c // 4, c % 4
        a0, a1 = 2 * j, 2 * j + 1
        g0 = 0 if j < 2 else 1

        xb = x[b]                                   # (S, D)
        xT3 = _chunk128(np.ascontiguousarray(xb.T))  # [128, 16, S]

        wq_sel = np.concatenate(
            [blk(Wq, a0), blk(Wq, a1), blk(Wq, a0 + 8), blk(Wq, a1 + 8)], axis=1)
        myblk = g0 + 2 * (j % 2)
        wk_sel = blk(Wk, myblk)
        wv_sel = blk(Wv, myblk)
        wo_sel = np.concatenate(
            [Wo[h * 128:(h + 1) * 128, :] for h in (a0, a1, a0 + 8, a1 + 8)],
            axis=0)

        cq_sel = _chunk128(np.ascontiguousarray(
            np.concatenate([blk(cos_q, a0), blk(cos_q, a1)], axis=1).T))
        sq_sel = _chunk128(np.ascontiguousarray(
            np.concatenate([blk(sin_q, a0), blk(sin_q, a1)], axis=1).T))
        ck_sel = np.ascontiguousarray(blk(cos_k, g0).T).astype(NPBF)
        sk_sel = np.ascontiguousarray(blk(sin_k, g0).T).astype(NPBF)

        in_maps.append({
            "xT": xT3,
            "wq": _chunk128(wq_sel),
            "wk": _chunk128(wk_sel),
            "wv": _chunk128(wv_sel),
            "wo": _chunk128(wo_sel),
            "cq": cq_sel, "sq": sq_sel, "ck": ck_sel, "sk": sk_sel,
        })

    global LAST_RESULT
    if _NC_CACHE is None:
        _NC_CACHE = _build_nc()
    res = run_bass_kernel_spmd(_NC_CACHE, in_maps, list(range(8)))
    LAST_RESULT = res

    partials = [
        res.results[c]["out"].astype(np.float32).transpose(1, 0, 2).reshape(S, D)
        for c in range(8)
    ]
    out = np.stack(
        [sum(partials[4 * b + j] for j in range(4)) for b in range(2)], axis=0
    )
    return (out + bo).astype(np.float32)
